# revision 2
# baseline (speedup 1.0000x reference)
"""Trainium2 Bass kernel for nn_CapsuleLayer (capsule layer w/ dynamic routing).

Math (reference):
    u_hat[b,c,u,s] = sum_p W[c,u,s,p] * X[b,p,c]
    b_ij = 0
    3x: c_ij = softmax_c(b_ij); s_j = sum_c c_ij*u_hat; v = squash_u(s_j)
        b_ij += mean_b sum_s u_hat*v
    return v[..., None]

Strategy: shard C=1152 across 8 cores (144 each); never materialize u_hat.
W is held per-core in a permuted [c, (p,u,s)] SBUF layout so every PE rhs
slice and DVE pass is contiguous. Matmul operands are bf16 (fp32 PSUM
accumulation, fp32 collectives and elementwise chain). Per iteration:
    A[c,(p,u,s)]   = W * e[c,u]                     (e = exp(b_ij), broadcast)
    s_raw[b,(u,s)] = sum_{p,ct} XT[p,ct].T @ A-slice  (PE, PSUM-accumulated)
    dp[u]          = sum_c e[c,u]                     (PE ones-matmul)
    collective over the 8 cores (AllReduce; ReduceScatter on the last
        iteration so each core only squashes its own 16 batch rows and the
        host assembles the output)
    s_j = s_raw/denom ; v = s_j * mag/(1+mag_sq)      (squash; mag over u)
    Z[c,(u,s)|p]   = X[:,p,:].T @ v                   (PE per p)
    agree[c,u]     = sum_{s,p} W * Z                  (TT + reduce, DVE)
    e *= exp(agree/B)
The collective payload is row-interleaved [row(160) | denom(10) | pad(6)]
so the summed denominator arrives per-partition (no broadcast needed) and
all staging is two uniform strided DMAs. GPSIMD only fires collective
triggers + a couple of DMAs; ACT runs only sqrt/exp with dummy ops keeping
both LUTs preloaded off the critical path. X transposes go through the
DMA xbar (bf16), keeping the PE free for matmuls.
"""

import numpy as np

import concourse.bass as bass
import concourse.mybir as mybir
import concourse.tile as tile
from concourse import bacc
from concourse.bass_utils import run_bass_kernel_spmd
from concourse.masks import make_identity

B, P, C, U, S = 128, 8, 1152, 10, 16
R = 3
NCORES = 8
CL = C // NCORES          # 144 local capsules
US = U * S                # 160
USP = US * P              # 1280
ROW = US + S              # 160 payload + 10 denom + 6 pad = 176 (32B-aligned)
BSL = B // NCORES         # 16 batch rows per core in the final scatter
CTS = [(0, 128), (128, CL - 128)]
F32 = mybir.dt.float32
BF16 = mybir.dt.bfloat16
ADD = mybir.AluOpType.add
MULT = mybir.AluOpType.mult
EXP = mybir.ActivationFunctionType.Exp


def _build_program():
    nc = bacc.Bacc("TRN2", target_bir_lowering=False, debug=False,
                   num_devices=NCORES)
    Xl = nc.dram_tensor("Xl", [B, P * CL], F32, kind="ExternalInput")
    Wl = nc.dram_tensor("Wl", [CL, USP], F32, kind="ExternalInput")
    Vout = nc.dram_tensor("Vout", [BSL, US], F32, kind="ExternalOutput")

    with tile.TileContext(nc) as tc:
        _emit(nc, tc, Xl, Wl, Vout)
    nc.compile()
    return nc


def _emit(nc, tc, Xl, Wl, Vout):
    rg = [list(range(NCORES))]
    with (
        tc.tile_pool(name="const", bufs=1) as cpool,
        tc.tile_pool(name="work", bufs=2) as wpool,
        tc.tile_pool(name="psum", bufs=1, space="PSUM") as ppool,
        tc.tile_pool(name="dram", bufs=2, space="DRAM") as dpool,
    ):
        # ---------------- warmup collective ----------------
        # the first collective on the CC cores pays ~50us of one-time
        # init/rendezvous; fire a tiny dummy AllReduce immediately so that
        # cost overlaps the setup DMAs + t=0 compute
        warm_in = dpool.tile([64], F32, tag="warmin")
        warm_out = dpool.tile([64], F32, tag="warmout")
        nc.gpsimd.collective_compute(
            "AllReduce", ADD, replica_groups=rg,
            ins=[warm_in[:].opt()], outs=[warm_out[:].opt()])

        # ---------------- constants / setup ----------------
        ones_col = [cpool.tile([sz, 1], F32, tag=f"ones{i}", name=f"ones{i}")
                    for i, (_, sz) in enumerate(CTS)]
        for t in ones_col:
            nc.vector.memset(t[:], 1.0)
        ones_row = cpool.tile([1, 128], F32, tag="onesrow")
        nc.vector.memset(ones_row[:], 1.0)
        scratch = cpool.tile([1, 4], F32, tag="scratch")
        nc.vector.memset(scratch[:], 1.0)
        # preload both ACT tables during setup
        nc.scalar.sqrt(scratch[:, 0:1], scratch[:, 1:2])
        nc.scalar.activation(scratch[:, 2:3], scratch[:, 3:4], EXP)

        X_sb = cpool.tile([B, P * CL], F32, tag="X")
        for q in range(4):
            eng = nc.sync if q % 2 == 0 else nc.scalar
            eng.dma_start(X_sb[:, q * 288:(q + 1) * 288],
                          Xl[:, q * 288:(q + 1) * 288])
        W_nat = [cpool.tile([sz, USP], F32, tag=f"Wn{i}", name=f"Wn{i}")
                 for i, (_, sz) in enumerate(CTS)]
        for i, (off, sz) in enumerate(CTS):
            nchunk = 4 if sz == 128 else 1
            step = USP // nchunk
            for k in range(nchunk):
                eng = nc.sync if (i + k) % 2 == 0 else nc.scalar
                eng.dma_start(W_nat[i][:, k * step:(k + 1) * step],
                              Wl[off:off + sz, k * step:(k + 1) * step])

        # X in bf16 (Z-matmul lhsT + transpose source)
        X16 = cpool.tile([B, P * CL], BF16, tag="X16")
        nc.vector.tensor_copy(X16[:, 0:576], X_sb[:, 0:576])
        nc.scalar.copy(X16[:, 576:1152], X_sb[:, 576:1152])
        # W permuted to [c, (p, u, s)]: bf16 (t=0 s_raw rhs — on the setup
        # critical path, so DVE takes the big tile) and fp32 (agree path,
        # not needed until mid-iteration 0)
        W_pm = [cpool.tile([sz, USP], F32, tag=f"W{i}", name=f"W{i}")
                for i, (_, sz) in enumerate(CTS)]
        W16 = [cpool.tile([sz, USP], BF16, tag=f"V{i}", name=f"V{i}")
               for i, (_, sz) in enumerate(CTS)]
        srcs = [W_nat[i][:].rearrange("q (u s p) -> q p u s", u=U, s=S, p=P)
                for i in range(2)]
        nc.vector.tensor_copy(
            W16[0][:].rearrange("q (p u s) -> q p u s", u=U, s=S, p=P),
            srcs[0])
        nc.scalar.copy(
            W16[1][:].rearrange("q (p u s) -> q p u s", u=U, s=S, p=P),
            srcs[1])
        nc.vector.tensor_copy(
            W_pm[0][:].rearrange("q (p u s) -> q p u s", u=U, s=S, p=P),
            srcs[0])
        nc.scalar.copy(
            W_pm[1][:].rearrange("q (p u s) -> q p u s", u=U, s=S, p=P),
            srcs[1])

        # XT[p][i] = transpose of X16[:, p, ctile_i] -> [c, b] via PE
        ident = cpool.tile([128, 128], BF16, tag="ident")
        make_identity(nc, ident[:])
        XT = [[cpool.tile([sz, 128], BF16, tag=f"XT{p}_{i}", name=f"XT{p}_{i}")
               for i, (_, sz) in enumerate(CTS)] for p in range(P)]
        with tc.tile_pool(name="ptr", bufs=2, space="PSUM") as ptrpool:
            for p in range(P):
                for i, (off, sz) in enumerate(CTS):
                    tp = ptrpool.tile([128, 128], BF16, tag="tr",
                                      name=f"tp{p}_{i}")
                    nc.tensor.transpose(
                        tp[:sz, :], X16[:, p * CL + off:p * CL + off + sz],
                        ident[:])
                    if (p + i) % 2 == 0:
                        nc.scalar.copy(XT[p][i][:], tp[:sz, :])
                    else:
                        nc.vector.tensor_copy(XT[p][i][:], tp[:sz, :])

        # setup used ACT COPY ops, which evict the activation LUTs; preload
        # sqrt+exp again afterwards (input chains off the last scalar copy)
        pre2 = cpool.tile([1, 2], F32, tag="pre2")
        nc.scalar.activation(pre2[:, 0:1], XT[7][1][0:1, 0:1], EXP)
        nc.scalar.sqrt(pre2[:, 1:2], pre2[:, 0:1])

        e_cur = [None, None]      # exp(b_ij) per c-tile; None => all-ones

        # allocated after the transpose pool frees its PSUM banks
        _pspool_cm = tc.tile_pool(name="psmall", bufs=4, space="PSUM")
        pspool = _pspool_cm.__enter__()
        for t in range(R):
            last = t == R - 1
            # ---------- A = W * e (bcast over p,s); skip at t=0 (e==1) -----
            if t == 0:
                A_sb = W16
            else:
                A_sb = [wpool.tile([sz, USP], BF16, tag=f"A{i}",
                                   name=f"A{i}_{t}")
                        for i, (_, sz) in enumerate(CTS)]
                for i in (1, 0):
                    sz = CTS[i][1]
                    eb = bass.AP(e_cur[i][:].tensor, e_cur[i][:].offset,
                                 [e_cur[i][:].ap[0], [0, P], [1, U], [0, S]])
                    nc.vector.tensor_tensor(
                        A_sb[i][:].rearrange("q (p u x) -> q p u x",
                                             p=P, u=U),
                        W_pm[i][:].rearrange("q (p u x) -> q p u x",
                                             p=P, u=U),
                        eb, MULT)

            # ---------- s_raw = sum_{p,ct} XT.T @ A[:, p-slice] ----------
            # tile0 matmuls first: they only need A[0], so the PE can start
            # while A[1] is still being formed
            s_ps = ppool.tile([B, US], F32, tag="big", name=f"sps{t}")
            n_mm = 2 * P
            k = 0
            for i in range(2):
                for p in range(P):
                    nc.tensor.matmul(
                        s_ps[:, :], XT[p][i][:],
                        A_sb[i][:, p * US:(p + 1) * US],
                        start=(k == 0), stop=(k == n_mm - 1))
                    k += 1

            # ---------- denominator partials (t>0; t=0 denom == C) --------
            if t > 0:
                dp_ps = pspool.tile([1, U], F32, tag="psm", name=f"dp{t}")
                for i in range(2):
                    nc.tensor.matmul(dp_ps[:, :], ones_col[i][:], e_cur[i][:],
                                     start=(i == 0), stop=(i == 1))
                # dp padded to 16 and broadcast across partitions via PE, so
                # the collective payload carries it per-row
                dp_pad = wpool.tile([1, S], F32, tag="dppad", name=f"dpp{t}")
                nc.vector.memset(dp_pad[:], 0.0)
                nc.vector.tensor_copy(dp_pad[:, 0:U], dp_ps[:, :])
                dpb_ps = pspool.tile([B, S], F32, tag="psm", name=f"dpb{t}")
                nc.tensor.matmul(dpb_ps[:, :], ones_row[:, :], dp_pad[:, :])
                dp_rep = wpool.tile([B, S], BF16 if not last else F32,
                                     tag=f"dprep{int(last)}", name=f"dpr{t}")
                nc.vector.tensor_copy(dp_rep[:, :], dpb_ps[:, :])

            # ---------- stage + collective ----------
            # payload rows: [s_raw row (160) | denom (10) | pad (6)]
            s_stage = wpool.tile([B, US], BF16 if not last else F32,
                                 tag=f"sstage{int(last)}", name=f"sstage{t}")
            nc.vector.tensor_copy(s_stage[:, :], s_ps[:, :])
            ccn = B * (ROW if t > 0 else US)
            ccdt = BF16 if not last else F32
            cc_in = dpool.tile([ccn], ccdt, tag=f"ccin{t}")
            cc_out = dpool.tile([ccn // (NCORES if last else 1)], ccdt,
                                tag=f"ccout{t}")
            rw = ROW if t > 0 else US
            nc.sync.dma_start(
                bass.AP(cc_in[:].tensor, cc_in[:].offset, [[rw, 64], [1, US]]),
                s_stage[0:64, :])
            nc.scalar.dma_start(
                bass.AP(cc_in[:].tensor, cc_in[:].offset + 64 * rw,
                        [[rw, 64], [1, US]]),
                s_stage[64:128, :])
            if t > 0:
                nc.gpsimd.dma_start(
                    bass.AP(cc_in[:].tensor, cc_in[:].offset + US,
                            [[ROW, B], [1, S]]),
                    dp_rep[:, :])
            nc.gpsimd.collective_compute(
                "AllReduce" if not last else "ReduceScatter", ADD,
                replica_groups=rg,
                ins=[cc_in[:].opt()], outs=[cc_out[:].opt()])

            nb = B if not last else BSL
            # gpsimd just came out of its collective wait, so it issues the
            # first read-back DMA with no engine-wake latency
            if last:
                s_in = wpool.tile([nb, rw], F32, tag="sinl", name=f"sin{t}")
                nc.gpsimd.dma_start(
                    s_in[:, :],
                    cc_out[:].rearrange("(b f) -> b f", b=nb))
            else:
                s_in16 = wpool.tile([nb, rw], BF16, tag="sin16",
                                    name=f"sin16_{t}")
                h = nb // 2
                nc.gpsimd.dma_start(
                    s_in16[0:h, :],
                    cc_out[0:h * rw].rearrange("(b f) -> b f", b=h))
                nc.sync.dma_start(
                    s_in16[h:nb, :],
                    cc_out[h * rw:].rearrange("(b f) -> b f", b=h))
                s_in = s_in16
            s_sum = s_in[:, 0:US]

            # ---------- s_j = s_sum / denom ----------
            s_j = wpool.tile([nb, US], F32, tag=f"sj{min(t, 1)}",
                             name=f"sj{t}")
            if t == 0:
                nc.vector.tensor_scalar_mul(s_j[:], s_sum, 1.0 / C)
            else:
                rd = wpool.tile([nb, U], F32, tag=f"rd{min(t, 1)}",
                                name=f"rd{t}")
                nc.vector.reciprocal(rd[:], s_in[:, US:US + U])
                nc.vector.tensor_tensor(
                    s_j[:].rearrange("q (u s) -> q u s", s=S),
                    s_sum.rearrange("q (u s) -> q u s", s=S),
                    rd[:].unsqueeze(2).broadcast_to((nb, U, S)), MULT)

            # ---------- v = squash(s_j): v = s_j * mag/(1+mag_sq) ----------
            sq = wpool.tile([nb, US], F32, tag=f"sq{min(t,1)}", name=f"sq{t}")
            nc.vector.tensor_tensor(sq[:], s_j[:], s_j[:], MULT)
            msq = wpool.tile([nb, S], F32, tag=f"msq{min(t,1)}",
                             name=f"msq{t}")
            nc.vector.tensor_reduce(
                msq[:], sq[:].rearrange("q (u s) -> q s u", u=U),
                axis=mybir.AxisListType.X, op=ADD)
            mag = wpool.tile([nb, S], F32, tag=f"mag{min(t,1)}",
                             name=f"mag{t}")
            nc.scalar.sqrt(mag[:], msq[:])
            if not last:
                # keep the EXP LUT resident for the upcoming e-update
                dex = wpool.tile([1, 1], F32, tag="dex", name=f"dex{t}")
                nc.scalar.activation(dex[:], mag[0:1, 0:1], EXP)
            h1 = wpool.tile([nb, S], F32, tag=f"h1{min(t,1)}", name=f"h1{t}")
            nc.vector.tensor_scalar_add(h1[:], msq[:], 1.0)
            rh = wpool.tile([nb, S], F32, tag=f"rh{min(t,1)}", name=f"rh{t}")
            nc.vector.reciprocal(rh[:], h1[:])
            g = wpool.tile([nb, S], F32, tag=f"g{min(t,1)}", name=f"g{t}")
            nc.vector.tensor_tensor(g[:], mag[:], rh[:], MULT)
            v_sb = wpool.tile([nb, US], F32 if last else BF16,
                              tag=f"v{min(t,1)}", name=f"v{t}")
            nc.vector.tensor_tensor(
                v_sb[:].rearrange("q (u s) -> q u s", s=S),
                s_j[:].rearrange("q (u s) -> q u s", s=S),
                g[:].unsqueeze(1).broadcast_to((nb, U, S)), MULT)

            if last:
                break

            # ---------- Z[c,(u,s)|p] = X[:,p,ctile].T @ v ----------
            # tile1 first: its staging copies + longer chain are critical
            z1_sb = wpool.tile([CTS[1][1], USP], BF16, tag="z1", name=f"z1{t}")
            for p in range(P):
                z1p = pspool.tile([CTS[1][1], US], F32, tag="psm",
                                  name=f"z1p{t}_{p}")
                nc.tensor.matmul(z1p[:, :],
                                 X16[:, p * CL + 128:(p + 1) * CL], v_sb[:])
                nc.vector.tensor_copy(z1_sb[:, p * US:(p + 1) * US],
                                      z1p[:, :])
            z_ps = ppool.tile([128, 2048], F32, tag="big", name=f"zps{t}")
            for p in range(P):
                nc.tensor.matmul(z_ps[:, p * 256:p * 256 + US],
                                 X16[:, p * CL:p * CL + 128], v_sb[:])

            # ---------- agree = sum_{s,p} W*Z ; e *= exp(agree/B) ----------
            e_new = [None, None]
            for i in (1, 0):
                sz = CTS[i][1]
                p1 = wpool.tile([sz, USP], F32 if i == 0 else BF16,
                                tag=f"p1_{i}", name=f"p1_{i}_{t}")
                if i == 0:
                    zv = bass.AP(z_ps[:].tensor, z_ps[:].offset,
                                 [z_ps[:].ap[0], [256, P], [1, US]])
                    nc.vector.tensor_tensor(
                        p1[:].rearrange("q (p x) -> q p x", p=P),
                        W_pm[i][:].rearrange("q (p x) -> q p x", p=P),
                        zv, MULT)
                else:
                    nc.vector.tensor_tensor(p1[:], W16[i][:], z1_sb[:], MULT)
                agr = wpool.tile([sz, U], F32, tag=f"agr{i}",
                                 name=f"agr{i}_{t}")
                pv = bass.AP(p1[:].tensor, p1[:].offset,
                             [p1[:].ap[0], [S, U], [US, P], [1, S]])
                nc.vector.tensor_reduce(agr[:], pv, axis=mybir.AxisListType.XY,
                                        op=ADD)
                eg = wpool.tile([sz, U], F32, tag=f"eg{i}", name=f"eg{i}_{t}")
                nc.scalar.activation(eg[:], agr[:], EXP, scale=1.0 / B)
                if e_cur[i] is None:
                    e_new[i] = eg
                else:
                    en = wpool.tile([sz, U], F32, tag=f"e{i}",
                                    name=f"e{i}_{t}")
                    nc.vector.tensor_tensor(en[:], e_cur[i][:], eg[:], MULT)
                    e_new[i] = en
            e_cur = e_new
            # preload SQRT table for next squash; the input depends on BOTH
            # e tiles so it always lands after the last exp call (otherwise
            # that exp reloads its LUT on the critical path)
            dd = wpool.tile([1, 1], F32, tag="dd", name=f"dd{t}")
            nc.vector.tensor_tensor(dd[:], e_cur[0][0:1, 0:1],
                                    e_cur[1][0:1, 0:1], MULT)
            dsq = wpool.tile([1, 1], F32, tag="dsq", name=f"dsq{t}")
            nc.scalar.sqrt(dsq[:], dd[:])

        # ---------------- output (this rank's batch slice) ----------------
        nc.sync.dma_start(Vout[:, :], v_sb[:, :])
        _pspool_cm.__exit__(None, None, None)


_NC_CACHE = None


def _get_program():
    global _NC_CACHE
    if _NC_CACHE is None:
        _NC_CACHE = _build_program()
    return _NC_CACHE


def kernel(X: np.ndarray, W: np.ndarray) -> np.ndarray:
    assert X.shape == (B, P, C) and W.shape == (C, U, S, P)
    nc = _get_program()
    in_maps = []
    for i in range(NCORES):
        sl = slice(i * CL, (i + 1) * CL)
        in_maps.append({
            "Xl": np.ascontiguousarray(
                X[:, :, sl], dtype=np.float32).reshape(B, P * CL),
            "Wl": np.ascontiguousarray(
                W[sl], dtype=np.float32).reshape(CL, USP),
        })
    res = run_bass_kernel_spmd(nc, in_maps, core_ids=list(range(NCORES)))
    out = np.empty((B, US), dtype=np.float32)
    for i in range(NCORES):
        out[i * BSL:(i + 1) * BSL] = res.results[i]["Vout"]
    return out.reshape(B, U, S, 1)



# revision 5
# speedup vs baseline: 1.3138x; 1.3138x over previous
"""Trainium2 Bass kernel for nn_CapsuleLayer (capsule layer w/ dynamic routing).

Math (reference):
    u_hat[b,c,u,s] = sum_p W[c,u,s,p] * X[b,p,c]
    b_ij = 0
    3x: c_ij = softmax_c(b_ij); s_j = sum_c c_ij*u_hat; v = squash_u(s_j)
        b_ij += mean_b sum_s u_hat*v
    return v[..., None]

Strategy: ZERO collectives. On this rig the first collective's mesh cannot
begin before ~72us after kernel start (CC-core boot + first-handshake
latency is fixed no matter when it is triggered), and each later AllReduce
costs ~13us, so any C- or B-sharded scheme is floored near ~140us. Instead
every core runs the FULL problem redundantly (engines are >80% idle in the
sharded version, so 8x redundant compute is cheap) and the host reads core
0's output. Inputs are pre-laid-out and pre-cast to bf16 on the host:
    XT[c,(p,b)]  - lhsT for the s_raw matmuls
    XB[b,(p,c)]  - lhsT for the Z (agreement) matmuls
    WF[c,(p,u,s)] - rhs/elementwise operand everywhere
Per routing iteration the per-c-tile pipeline is
    Z[c,(p,u,s)] = XB_p.T @ v          (PE, 8 matmuls -> PSUM fp32)
    p1 = WF * Z                        (DVE, 1x due to fp32 PSUM operand)
    q1 = fold_p(p1); q2 = fold_p(q1)   (GPSIMD adds)
    agr[c,u] = reduce_(p2,s) q2        (DVE)
    b += agr; es = exp(b/B)            (GPSIMD add, ACT exp -> bf16)
    A = WF * es                        (DVE, 2x all-bf16)
    s_raw += XT_kp.T @ A               (PE, accumulated over all (k,p))
with the squash + softmax denominator (PE ones-matmuls + reciprocal +
PE broadcast-matmul) between iterations. ACT LUT swaps (exp<->sqrt) are
forced off the critical path with data-chained dummy ops.
"""

import numpy as np
import ml_dtypes

import concourse.bass as bass
import concourse.mybir as mybir
import concourse.tile as tile
from concourse import bacc
from concourse.bass_utils import run_bass_kernel_spmd

B, P, C, U, S = 128, 8, 1152, 10, 16
R = 3
NCORES = 8
NT = C // 128            # 9 c-tiles
US = U * S               # 160
USP = US * P             # 1280
PB = P * B               # 1024
INV_B = 1.0 / B
F32 = mybir.dt.float32
BF16 = mybir.dt.bfloat16
ADD = mybir.AluOpType.add
MULT = mybir.AluOpType.mult
EXP = mybir.ActivationFunctionType.Exp
XY = mybir.AxisListType.XY
X_AX = mybir.AxisListType.X
SKEW = 3                 # s_raw matmuls trail the Z matmuls by this many tiles


def _build_program():
    nc = bacc.Bacc("TRN2", target_bir_lowering=False, debug=False,
                   num_devices=NCORES)
    XT = nc.dram_tensor("XT", [C, PB], BF16, kind="ExternalInput")
    XB = nc.dram_tensor("XB", [B, P * C], BF16, kind="ExternalInput")
    WF = nc.dram_tensor("WF", [C, USP], BF16, kind="ExternalInput")
    VO = nc.dram_tensor("VO", [B, US], F32, kind="ExternalOutput")

    with tile.TileContext(nc) as tc:
        _emit(nc, tc, XT, XB, WF, VO)
    nc.compile()
    return nc


def _squash(nc, wpool, s_ps, rd_rep, t, last):
    """v = squash(s_raw / denom). rd_rep is [128,U] f32 (None at t=0)."""
    s_j = wpool.tile([B, US], F32, tag=f"sj{min(t, 1)}", name=f"sj{t}")
    if rd_rep is None:
        nc.vector.tensor_scalar_mul(s_j[:], s_ps[:], 1.0 / C)
    else:
        nc.vector.tensor_tensor(
            s_j[:].rearrange("q (u s) -> q u s", s=S),
            s_ps[:].rearrange("q (u s) -> q u s", s=S),
            rd_rep[:].unsqueeze(2).broadcast_to((B, U, S)), MULT)
    sq = wpool.tile([B, US], F32, tag=f"sq{min(t, 1)}", name=f"sq{t}")
    nc.vector.tensor_tensor(sq[:], s_j[:], s_j[:], MULT)
    msq = wpool.tile([B, S], F32, tag=f"msq{min(t, 1)}", name=f"msq{t}")
    nc.vector.tensor_reduce(
        msq[:], sq[:].rearrange("q (u s) -> q s u", u=U),
        axis=X_AX, op=ADD)
    mag = wpool.tile([B, S], F32, tag=f"mag{min(t, 1)}", name=f"mag{t}")
    nc.scalar.sqrt(mag[:], msq[:])
    h1 = wpool.tile([B, S], F32, tag=f"h1{min(t, 1)}", name=f"h1{t}")
    nc.vector.tensor_scalar_add(h1[:], msq[:], 1.0)
    rh = wpool.tile([B, S], F32, tag=f"rh{min(t, 1)}", name=f"rh{t}")
    nc.vector.reciprocal(rh[:], h1[:])
    g = wpool.tile([B, S], F32, tag=f"g{min(t, 1)}", name=f"g{t}")
    nc.vector.tensor_tensor(g[:], mag[:], rh[:], MULT)
    v_sb = wpool.tile([B, US], F32 if last else BF16,
                      tag=f"v{min(t, 1)}", name=f"v{t}")
    nc.vector.tensor_tensor(
        v_sb[:].rearrange("q (u s) -> q u s", s=S),
        s_j[:].rearrange("q (u s) -> q u s", s=S),
        g[:].unsqueeze(1).broadcast_to((B, U, S)), MULT)
    return v_sb, mag


def _emit(nc, tc, XT, XB, WF, VO):
    with (
        tc.tile_pool(name="const", bufs=1) as cpool,
        tc.tile_pool(name="work", bufs=2) as wpool,
        tc.tile_pool(name="amat", bufs=4) as apool,
        tc.tile_pool(name="bstate", bufs=2) as bpool,
        tc.tile_pool(name="zps", bufs=1, space="PSUM") as zpool,
        tc.tile_pool(name="sps", bufs=2, space="PSUM") as spool,
        tc.tile_pool(name="smallps", bufs=1, space="PSUM") as pspool,
    ):
        # ---------------- constants ----------------
        ones_bf = cpool.tile([128, 1], BF16, tag="onesb")
        nc.gpsimd.memset(ones_bf[:], 1.0)
        ones_row = cpool.tile([1, 128], F32, tag="onesr")
        nc.gpsimd.memset(ones_row[:], 1.0)
        # preload the SQRT LUT now (needed first at squash0); EXP comes later
        scr = cpool.tile([1, 2], F32, tag="scr")
        nc.gpsimd.memset(scr[:], 1.0)
        pre = cpool.tile([1, 1], F32, tag="pre")
        nc.scalar.sqrt(pre[:], scr[:, 0:1])

        # ---------------- input DMA (round-robin over 5 queues) ----------
        Wt = [cpool.tile([128, USP], BF16, tag=f"W{k}", name=f"W{k}")
              for k in range(NT)]
        XTt = [cpool.tile([128, PB], BF16, tag=f"XT{k}", name=f"XTk{k}")
               for k in range(NT)]
        XBt = [cpool.tile([B, C], BF16, tag=f"XB{p}", name=f"XBp{p}")
               for p in range(P)]
        transfers = []
        for k in range(3):
            transfers.append((Wt[k], WF, k))
            transfers.append((XTt[k], XT, k))
        for k in range(3, NT):
            transfers.append((XBt[k - 3], XB, k - 3))
            transfers.append((Wt[k], WF, k))
            transfers.append((XTt[k], XT, k))
        transfers.append((XBt[6], XB, 6))
        transfers.append((XBt[7], XB, 7))
        qs = [nc.sync, nc.scalar, nc.gpsimd]
        for i, (dst, src, k) in enumerate(transfers):
            eng = qs[i % len(qs)]
            if src is XB:
                eng.dma_start(dst[:, :], XB[:, k * C:(k + 1) * C])
            else:
                eng.dma_start(dst[:, :], src[k * 128:(k + 1) * 128, :])

        # ---------------- t = 0: s_raw = sum W (c_ij uniform) -------------
        s_ps = spool.tile([B, US], F32, tag="s", name="sps0")
        for k in range(NT):
            for p in range(P):
                nc.tensor.matmul(
                    s_ps[:, :], XTt[k][:, p * 128:(p + 1) * 128],
                    Wt[k][:, p * US:(p + 1) * US],
                    start=(k == 0 and p == 0),
                    stop=(k == NT - 1 and p == P - 1))

        b_cur = [None] * NT
        v16, mag0 = _squash(nc, wpool, s_ps, None, 0, last=False)
        # load the EXP LUT while the first Z matmuls run (chained on mag)
        dex = wpool.tile([1, 1], F32, tag="dex", name="dex0")
        nc.scalar.activation(dex[:], mag0[0:1, 0:1], EXP)

        # ---------------- routing iterations ------------------------------
        for t in range(R - 1):
            last = t == R - 2
            s_nxt = spool.tile([B, US], F32, tag="s", name=f"sps{t + 1}")
            den_ps = pspool.tile([1, U], F32, tag="den", name=f"den{t}")
            A_l = [None] * NT
            es_l = [None] * NT
            for k in range(NT + SKEW):
                if k < NT:
                    z = zpool.tile([128, 2048], F32, tag="z", name=f"z{t}_{k}")
                    for p in range(P):
                        nc.tensor.matmul(
                            z[:, p * 256:p * 256 + US],
                            XBt[p][:, k * 128:(k + 1) * 128], v16[:, :])
                    p1 = wpool.tile([128, USP], BF16, tag="p1",
                                    name=f"p1_{t}_{k}")
                    zv = bass.AP(z[:].tensor, z[:].offset,
                                 [z[:].ap[0], [256, P], [1, US]])
                    nc.vector.tensor_tensor(
                        p1[:].rearrange("q (p f) -> q p f", p=P),
                        Wt[k][:].rearrange("q (p f) -> q p f", p=P),
                        zv, MULT)
                    q1 = wpool.tile([128, USP // 2], BF16, tag="q1",
                                    name=f"q1_{t}_{k}")
                    nc.gpsimd.tensor_tensor(q1[:], p1[:, 0:640],
                                            p1[:, 640:1280], ADD)
                    q2 = wpool.tile([128, USP // 4], BF16, tag="q2",
                                    name=f"q2_{t}_{k}")
                    nc.gpsimd.tensor_tensor(q2[:], q1[:, 0:320],
                                            q1[:, 320:640], ADD)
                    rv = bass.AP(q2[:].tensor, q2[:].offset,
                                 [q2[:].ap[0], [S, U], [US, 2], [1, S]])
                    bt = bpool.tile([128, U], F32, tag=f"b{k}",
                                    name=f"b{t}_{k}")
                    if t == 0:
                        nc.vector.tensor_reduce(bt[:], rv, axis=XY, op=ADD)
                    else:
                        agr = wpool.tile([128, U], F32, tag="agr",
                                         name=f"agr{t}_{k}")
                        nc.vector.tensor_reduce(agr[:], rv, axis=XY, op=ADD)
                        nc.gpsimd.tensor_tensor(bt[:], b_cur[k][:], agr[:],
                                                ADD)
                    b_cur[k] = bt
                    es = wpool.tile([128, US], BF16, tag="es",
                                    name=f"es{t}_{k}")
                    nc.scalar.activation(
                        es[:].rearrange("q (u s) -> q u s", s=S),
                        bt[:].unsqueeze(2).broadcast_to((128, U, S)),
                        EXP, scale=INV_B)
                    es_l[k] = es
                    A = apool.tile([128, USP], BF16, tag="A",
                                   name=f"A{t}_{k}")
                    nc.vector.tensor_tensor(
                        A[:].rearrange("q (p f) -> q p f", p=P),
                        Wt[k][:].rearrange("q (p f) -> q p f", p=P),
                        es[:].unsqueeze(1).broadcast_to((128, P, US)), MULT)
                    A_l[k] = A
                ks = k - SKEW
                if ks >= 0:
                    for p in range(P):
                        nc.tensor.matmul(
                            s_nxt[:, :], XTt[ks][:, p * 128:(p + 1) * 128],
                            A_l[ks][:, p * US:(p + 1) * US],
                            start=(ks == 0 and p == 0),
                            stop=(ks == NT - 1 and p == P - 1))

            # swap in the SQRT LUT once the last exp of this iter is issued
            dsq = wpool.tile([1, 1], F32, tag="dsq", name=f"dsq{t}")
            nc.scalar.sqrt(dsq[:], es_l[NT - 1][0:1, 0:1])

            # softmax denominator: den[u] = sum_c es; rd_rep = 1/den bcast
            for k in range(NT):
                eap = bass.AP(es_l[k][:].tensor, es_l[k][:].offset,
                              [es_l[k][:].ap[0], [S, U]])
                nc.tensor.matmul(den_ps[:, :], ones_bf[:, :], eap,
                                 start=(k == 0), stop=(k == NT - 1))
            rdv = wpool.tile([1, U], F32, tag="rdv", name=f"rdv{t}")
            nc.vector.reciprocal(rdv[:], den_ps[:, :])
            rdb_ps = pspool.tile([128, U], F32, tag="rdb", name=f"rdb{t}")
            nc.tensor.matmul(rdb_ps[:, :], ones_row[:, :], rdv[:, :])
            rd_rep = wpool.tile([128, U], F32, tag="rdrep", name=f"rdr{t}")
            nc.vector.tensor_copy(rd_rep[:], rdb_ps[:, :])

            v16, magt = _squash(nc, wpool, s_nxt, rd_rep, t + 1, last=last)
            if not last:
                dex2 = wpool.tile([1, 1], F32, tag="dex", name=f"dex{t + 1}")
                nc.scalar.activation(dex2[:], magt[0:1, 0:1], EXP)

        # ---------------- output ------------------------------------------
        nc.sync.dma_start(VO[:, :], v16[:, :])


_NC_CACHE = None


def _get_program():
    global _NC_CACHE
    if _NC_CACHE is None:
        _NC_CACHE = _build_program()
    return _NC_CACHE


def make_in_maps(X: np.ndarray, W: np.ndarray) -> list[dict]:
    bf = ml_dtypes.bfloat16
    XTn = np.ascontiguousarray(
        np.asarray(X, dtype=np.float32).transpose(2, 1, 0)).astype(
            bf).reshape(C, PB)
    XBn = np.ascontiguousarray(
        np.asarray(X, dtype=np.float32).reshape(B, P * C)).astype(bf)
    WFn = np.ascontiguousarray(
        np.asarray(W, dtype=np.float32).transpose(0, 3, 1, 2)).astype(
            bf).reshape(C, USP)
    im = {"XT": XTn, "XB": XBn, "WF": WFn}
    return [im for _ in range(NCORES)]


def kernel(X: np.ndarray, W: np.ndarray) -> np.ndarray:
    assert X.shape == (B, P, C) and W.shape == (C, U, S, P)
    nc = _get_program()
    res = run_bass_kernel_spmd(nc, make_in_maps(X, W),
                               core_ids=list(range(NCORES)))
    out = np.asarray(res.results[0]["VO"], dtype=np.float32)
    return out.reshape(B, U, S, 1)
